# revision 20
# baseline (speedup 1.0000x reference)
"""Dependency-GAN message-passing kernel for 8 Trainium2 NeuronCores.

Reference computation (B=2, L=384, H=768, D=H/2=384, EMB=64, NTYPES=76):
  dep_graphs = scatter-add COO edge types into dense [B,L,L] int32
  dep_masks  = (1 - (dep_graphs>0)) * -10000
  q = hidden @ W_q.T + b_q ; k = hidden @ W_k.T + b_k          [B,L,D]
  elmwise = q[:,:,None,:] * k[:,None,:,:]                      [B,L,L,D]
  scores  = elmwise @ w_att[:D] + etype_emb[dep_graphs] @ w_att[D:] + b_att
  probs   = softmax(scores + dep_masks)
  output  = probs @ hidden                                     [B,L,H]
returns (output, elmwise, dep_graphs, dep_masks)

Sharding: data-parallel over batch (4 cores per batch element) x
sequence-parallel over query rows (96 rows per core). Each core computes its
[96,L,D] slice of elmwise (the dominant ~54 MiB DRAM write), its scores rows,
softmax, and its [96,H] slice of the attention output.

The tiny data-dependent parts (COO scatter with duplicate summation, the
76-entry edge-score table lookup) run on host; everything O(L^2) runs on
device. The edge-type embedding never needs materialising: its score
contribution is proj[dep_graphs] with proj = etype_emb @ w_att[D:].
"""

import os
import sys

import numpy as np

for _p in ("/opt/trn_rl_repo",):
    if _p not in sys.path and os.path.isdir(_p):
        sys.path.insert(0, _p)

B, L, H = 2, 384, 768
D = H // 2
EMB = 64
NTYPES = 76
NCORES = 8
CORES_PER_B = NCORES // B          # 4
ILEN = L // CORES_PER_B            # 96 query rows per core
KT = H // 128                      # 6 contraction tiles over h
DT = D // 128                      # 3 tiles over d
JT = L // 128                      # 3 tiles over j (key rows)
CHUNK = 12                         # query rows per q_flat chunk
NCHUNK = ILEN // CHUNK             # 8
IB = 4                             # query rows batched per elmwise DMA

_CACHE = {}


def _build_bass(repeat=1, mode="full"):
    import concourse.bass as bass
    import concourse.tile as tile
    from concourse import bacc, mybir
    from concourse.masks import make_identity
    from contextlib import ExitStack

    f32 = mybir.dt.float32
    nc = bacc.Bacc("TRN2", target_bir_lowering=False)

    # ---- DRAM I/O (per-core tensors; same shapes on every core) ----
    hT = nc.dram_tensor("hT", [H, L], f32, kind="ExternalInput")        # hidden[b].T
    hTq = nc.dram_tensor("hTq", [H, ILEN], f32, kind="ExternalInput")   # hT[:, i-slice]
    hB = nc.dram_tensor("hB", [L, H], f32, kind="ExternalInput")        # hidden[b]
    WqT = nc.dram_tensor("WqT", [H, D], f32, kind="ExternalInput")
    WkT = nc.dram_tensor("WkT", [H, D], f32, kind="ExternalInput")
    bq = nc.dram_tensor("bq", [1, D], f32, kind="ExternalInput")
    bk = nc.dram_tensor("bk", [1, D], f32, kind="ExternalInput")
    wd = nc.dram_tensor("wd", [D, 1], f32, kind="ExternalInput")        # w_att[:D]
    sbias = nc.dram_tensor("sbias", [ILEN, L], f32, kind="ExternalInput")
    elm_out = nc.dram_tensor("elm_out", [ILEN, L, D], f32, kind="ExternalOutput")
    av_out = nc.dram_tensor("av_out", [ILEN, H], f32, kind="ExternalOutput")
    q_scr = nc.dram_tensor("q_scr", [1, ILEN * D], f32, kind="Internal")

    with ExitStack() as ctx:
        tc = ctx.enter_context(tile.TileContext(nc))
        const = ctx.enter_context(tc.tile_pool(name="const", bufs=1))
        sbwork = ctx.enter_context(tc.tile_pool(name="sbwork", bufs=1))
        qfpool = ctx.enter_context(tc.tile_pool(name="qf", bufs=2))
        elmpool = ctx.enter_context(tc.tile_pool(name="elm", bufs=3))
        mmps = ctx.enter_context(tc.tile_pool(name="mmps", bufs=2, space="PSUM"))
        repps = ctx.enter_context(tc.tile_pool(name="repps", bufs=3, space="PSUM"))
        avps = ctx.enter_context(tc.tile_pool(name="avps", bufs=2, space="PSUM"))
        scps = ctx.enter_context(tc.tile_pool(name="scps", bufs=1, space="PSUM"))

        # ---- constant loads ----
        hT_sb = const.tile([128, KT, L], f32)
        nc.sync.dma_start(out=hT_sb, in_=hT[:].rearrange("(t p) l -> p t l", p=128))
        hTq_sb = const.tile([128, KT, ILEN], f32)
        nc.sync.dma_start(out=hTq_sb, in_=hTq[:].rearrange("(t p) l -> p t l", p=128))
        hB_sb = const.tile([128, JT, H], f32)
        nc.sync.dma_start(out=hB_sb, in_=hB[:].rearrange("(t p) h -> p t h", p=128))
        WqT_sb = const.tile([128, KT, D], f32)
        nc.sync.dma_start(out=WqT_sb, in_=WqT[:].rearrange("(t p) d -> p t d", p=128))
        WkT_sb = const.tile([128, KT, D], f32)
        nc.sync.dma_start(out=WkT_sb, in_=WkT[:].rearrange("(t p) d -> p t d", p=128))
        bq_sb = const.tile([1, D], f32)
        nc.sync.dma_start(out=bq_sb, in_=bq[:])
        bk_sb = const.tile([1, D], f32)
        nc.sync.dma_start(out=bk_sb, in_=bk[:])
        wd_sb = const.tile([128, DT], f32)
        nc.sync.dma_start(out=wd_sb, in_=wd[:].rearrange("(t p) one -> p (t one)", p=128))
        sbias_sb = const.tile([ILEN, L], f32)
        nc.sync.dma_start(out=sbias_sb, in_=sbias[:])
        ones_sb = const.tile([1, L], f32)
        nc.vector.memset(ones_sb, 1.0)
        ident = const.tile([128, 128], f32)
        make_identity(nc, ident)

        def bcast(ap, reps):
            import concourse.bass as bass_mod
            return bass_mod.AP(
                tensor=ap.tensor,
                offset=ap.offset,
                ap=[list(ap.ap[0]), [0, reps]] + [list(a) for a in ap.ap[1:]],
            )

        def body():
            # ---- q rows for this core's i-slice: q_rd[i, d] (i on partitions) ----
            q_ps = mmps.tile([ILEN, D], f32, tag="mm")
            for t in range(KT):
                nc.tensor.matmul(q_ps, lhsT=hTq_sb[:, t, :], rhs=WqT_sb[:, t, :],
                                 start=(t == 0), stop=False)
            nc.tensor.matmul(q_ps, lhsT=ones_sb[0:1, 0:ILEN], rhs=bq_sb,
                             start=False, stop=True)
            q_rd = sbwork.tile([ILEN, D], f32, tag="q_rd")
            nc.vector.tensor_copy(q_rd, q_ps)
            # round-trip q through DRAM to get a flat single-partition copy
            nc.sync.dma_start(out=q_scr[0, :].rearrange("(i d) -> i d", d=D),
                              in_=q_rd)

            # ---- k rows for the elementwise stage ----
            # Interleaved layout: partition p, slot r holds k[3p + r, :] so a
            # [128, IB, 3, D] elm tile maps to one contiguous DRAM run per
            # partition (j-major order j = 3p + r).
            k_rd = sbwork.tile([128, JT, D], f32, tag="k_rd")
            for t in range(JT):
                k_ps = mmps.tile([128, D], f32, tag="mm")
                for u in range(KT):
                    lhsT = hT_sb[:, u, :].rearrange("p (j r) -> p r j", r=JT)[:, t, :]
                    nc.tensor.matmul(k_ps, lhsT=lhsT,
                                     rhs=WkT_sb[:, u, :], start=(u == 0), stop=False)
                nc.tensor.matmul(k_ps, lhsT=ones_sb[0:1, 0:128], rhs=bk_sb,
                                 start=False, stop=True)
                nc.vector.tensor_copy(k_rd[:, t, :], k_ps)

            # ---- qT (scaled by w_att[:D]) and kT, [d, row] layout for scores ----
            qsT = sbwork.tile([128, DT, ILEN], f32, tag="qsT")
            for t in range(DT):
                qT_ps = mmps.tile([128, ILEN], f32, tag="mm")
                for u in range(KT):
                    nc.tensor.matmul(qT_ps,
                                     lhsT=WqT_sb[:, u, t * 128:(t + 1) * 128],
                                     rhs=hTq_sb[:, u, :], start=(u == 0), stop=False)
                nc.tensor.matmul(qT_ps, lhsT=bq_sb[0:1, t * 128:(t + 1) * 128],
                                 rhs=ones_sb[0:1, 0:ILEN], start=False, stop=True)
                nc.vector.tensor_scalar_mul(qsT[:, t, :], in0=qT_ps,
                                            scalar1=wd_sb[:, t:t + 1])
            kT = sbwork.tile([128, DT, L], f32, tag="kT")
            for t in range(DT):
                kT_ps = mmps.tile([128, L], f32, tag="mm")
                for u in range(KT):
                    nc.tensor.matmul(kT_ps,
                                     lhsT=WkT_sb[:, u, t * 128:(t + 1) * 128],
                                     rhs=hT_sb[:, u, :], start=(u == 0), stop=False)
                nc.tensor.matmul(kT_ps, lhsT=bk_sb[0:1, t * 128:(t + 1) * 128],
                                 rhs=ones_sb[0:1, 0:L], start=False, stop=True)
                nc.vector.tensor_copy(kT[:, t, :], kT_ps)

            # ---- scores, softmax ----
            sc_ps = scps.tile([ILEN, L], f32, tag="sc")
            for t in range(DT):
                nc.tensor.matmul(sc_ps, lhsT=qsT[:, t, :], rhs=kT[:, t, :],
                                 start=(t == 0), stop=(t == DT - 1))
            probs = sbwork.tile([ILEN, L], f32, tag="probs")
            nc.vector.tensor_add(out=probs, in0=sc_ps, in1=sbias_sb)
            negmax = sbwork.tile([ILEN, 1], f32, tag="negmax")
            nc.vector.reduce_max(out=negmax, in_=probs, axis=mybir.AxisListType.X,
                                 negate=True)
            sums = sbwork.tile([ILEN, 1], f32, tag="sums")
            nc.scalar.activation(out=probs, in_=probs,
                                 func=mybir.ActivationFunctionType.Exp,
                                 bias=negmax, scale=1.0, accum_out=sums)
            rsum = sbwork.tile([ILEN, 1], f32, tag="rsum")
            nc.vector.reciprocal(out=rsum, in_=sums)
            nc.vector.tensor_scalar_mul(out=probs, in0=probs, scalar1=rsum)

            # ---- probs.T via PE transpose, then output = probs @ hidden ----
            pT = sbwork.tile([128, JT, ILEN], f32, tag="pT")
            for t in range(JT):
                pt_ps = mmps.tile([128, ILEN], f32, tag="mm")
                nc.tensor.transpose(pt_ps, probs[:, t * 128:(t + 1) * 128],
                                    ident[0:ILEN, 0:ILEN])
                nc.vector.tensor_copy(pT[:, t, :], pt_ps)
            av_sb = sbwork.tile([ILEN, H], f32, tag="av_sb")
            for half in range(2):
                av_ps = avps.tile([ILEN, 384], f32, tag="av")
                for t in range(JT):
                    nc.tensor.matmul(av_ps, lhsT=pT[:, t, :],
                                     rhs=hB_sb[:, t, half * 384:(half + 1) * 384],
                                     start=(t == 0), stop=(t == JT - 1))
                nc.vector.tensor_copy(av_sb[:, half * 384:(half + 1) * 384], av_ps)
            nc.sync.dma_start(out=av_out[:], in_=av_sb)

            # ---- the big one: elmwise[i, j, d] = q[i, d] * k[j, d] ----
            for c in range(NCHUNK):
                q_flat = qfpool.tile([1, CHUNK * D], f32, tag="qf")
                nc.sync.dma_start(out=q_flat,
                                  in_=q_scr[0:1, c * CHUNK * D:(c + 1) * CHUNK * D])
                for g in range(CHUNK // IB):
                    em = (elmpool.tile([128, IB, JT, D], f32, tag="em", name="em")
                          if mode != "nodve" else None)
                    for ib in range(IB):
                        ii = g * IB + ib
                        i = c * CHUNK + ii
                        rep = repps.tile([128, D], f32, tag="rep")
                        nc.tensor.matmul(rep, lhsT=ones_sb[0:1, 0:128],
                                         rhs=q_flat[0:1, ii * D:(ii + 1) * D],
                                         start=True, stop=True)
                        if mode != "nodve":
                            nc.vector.tensor_mul(out=em[:, ib], in0=k_rd,
                                                 in1=bcast(rep, JT))
                    if mode == "nodma":
                        continue
                    i0g = c * CHUNK + g * IB
                    dst = elm_out[i0g:i0g + IB].rearrange(
                        "ii (p r) d -> p ii r d", r=JT)
                    src = bcast(k_rd, IB) if mode == "nodve" else em
                    nc.sync.dma_start(out=dst, in_=src)

        for _rep in range(repeat):
            body()

    nc.finalize()
    return nc


def _get_nc(repeat=1, mode="full"):
    key = f"nc_{repeat}_{mode}"
    if key not in _CACHE:
        _CACHE[key] = _build_bass(repeat, mode)
    return _CACHE[key]


def _host_prep(**inputs):
    hidden = np.ascontiguousarray(np.asarray(inputs["hidden"]), dtype=np.float32)
    coo = np.asarray(inputs["dep_graph_coo"])
    etype = np.asarray(inputs["dep_graph_etype"])
    W_q = np.asarray(inputs["W_q"], dtype=np.float32)
    b_q = np.asarray(inputs["b_q"], dtype=np.float32)
    W_k = np.asarray(inputs["W_k"], dtype=np.float32)
    b_k = np.asarray(inputs["b_k"], dtype=np.float32)
    w_att = np.asarray(inputs["w_att"], dtype=np.float32)
    b_att = np.float32(np.asarray(inputs["b_att"]))
    etype_emb = np.asarray(inputs["etype_emb"], dtype=np.float32)

    # host: COO -> dense with duplicate summation (tiny, data-dependent)
    dep_graphs = np.zeros((B, L, L), dtype=np.int32)
    for b in range(B):
        np.add.at(dep_graphs[b],
                  (coo[b, 0].astype(np.int64), coo[b, 1].astype(np.int64)),
                  etype[b].astype(np.int32))
    dep_masks = (1.0 - (dep_graphs > 0)).astype(np.float32) * np.float32(-10000.0)
    # edge-type score contribution collapses to a 76-entry table lookup
    proj = (etype_emb @ w_att[D:]).astype(np.float32)
    edge_sc = np.take(proj, np.clip(dep_graphs, 0, NTYPES - 1))
    score_bias = (edge_sc + dep_masks + b_att).astype(np.float32)

    WqT = np.ascontiguousarray(W_q.T)
    WkT = np.ascontiguousarray(W_k.T)
    wd_col = np.ascontiguousarray(w_att[:D, None])
    bq_row = np.ascontiguousarray(b_q[None, :])
    bk_row = np.ascontiguousarray(b_k[None, :])

    in_maps = []
    for c in range(NCORES):
        b, i0 = c // CORES_PER_B, (c % CORES_PER_B) * ILEN
        hb = hidden[b]
        hbT = np.ascontiguousarray(hb.T)
        in_maps.append({
            "hT": hbT,
            "hTq": np.ascontiguousarray(hbT[:, i0:i0 + ILEN]),
            "hB": hb,
            "WqT": WqT,
            "WkT": WkT,
            "bq": bq_row,
            "bk": bk_row,
            "wd": wd_col,
            "sbias": np.ascontiguousarray(score_bias[b, i0:i0 + ILEN]),
        })
    return in_maps, dep_graphs, dep_masks


def make_in_maps(**inputs):
    return _host_prep(**inputs)[0]


def kernel(**inputs):
    in_maps, dep_graphs, dep_masks = _host_prep(**inputs)

    from concourse import bass_utils
    nc = _get_nc()
    res = bass_utils.run_bass_kernel_spmd(nc, in_maps, core_ids=list(range(NCORES)))
    _CACHE["last_results"] = res

    output = np.empty((B, L, H), dtype=np.float32)
    elmwise = np.empty((B, L, L, D), dtype=np.float32)
    for c in range(NCORES):
        b, i0 = c // CORES_PER_B, (c % CORES_PER_B) * ILEN
        output[b, i0:i0 + ILEN] = res.results[c]["av_out"]
        elmwise[b, i0:i0 + ILEN] = res.results[c]["elm_out"]
    return output, elmwise, dep_graphs, dep_masks


# revision 32
# speedup vs baseline: 2.0436x; 2.0436x over previous
"""Dependency-GAN message-passing kernel for 8 Trainium2 NeuronCores.

Reference computation (B=2, L=384, H=768, D=H/2=384, EMB=64, NTYPES=76):
  dep_graphs = scatter-add COO edge types into dense [B,L,L] int32
  dep_masks  = (1 - (dep_graphs>0)) * -10000
  q = hidden @ W_q.T + b_q ; k = hidden @ W_k.T + b_k          [B,L,D]
  elmwise = q[:,:,None,:] * k[:,None,:,:]                      [B,L,L,D]
  scores  = elmwise @ w_att[:D] + etype_emb[dep_graphs] @ w_att[D:] + b_att
  probs   = softmax(scores + dep_masks)
  output  = probs @ hidden                                     [B,L,H]
returns (output, elmwise, dep_graphs, dep_masks)

Sharding: data-parallel over batch (4 cores per batch element) x
sequence-parallel over query rows (96 rows per core). Each core computes its
[96,L,D] slice of elmwise (the dominant ~54 MiB DRAM write), its scores rows,
softmax, and its [96,H] slice of the attention output.

The tiny data-dependent parts (COO scatter with duplicate summation, the
76-entry edge-score table lookup) run on host; everything O(L^2) runs on
device. The edge-type embedding never needs materialising: its score
contribution is proj[dep_graphs] with proj = etype_emb @ w_att[D:].
"""

import os
import sys

import numpy as np

for _p in ("/opt/trn_rl_repo",):
    if _p not in sys.path and os.path.isdir(_p):
        sys.path.insert(0, _p)

B, L, H = 2, 384, 768
D = H // 2
EMB = 64
NTYPES = 76
NCORES = 8
CORES_PER_B = NCORES // B          # 4
ILEN = L // CORES_PER_B            # 96 query rows per core
KT = H // 128                      # 6 contraction tiles over h
DT = D // 128                      # 3 tiles over d
JT = L // 128                      # 3 tiles over j (key rows)
CHUNK = 12                         # query rows per q_flat chunk
NCHUNK = ILEN // CHUNK             # 8
IB = 4                             # query rows batched per elmwise DMA

_CACHE = {}


def _build_bass(repeat=1, mode="full", elm="psum3"):
    import concourse.bass as bass
    import concourse.tile as tile
    from concourse import bacc, mybir
    from concourse.masks import make_identity
    from contextlib import ExitStack

    f32 = mybir.dt.float32
    nc = bacc.Bacc("TRN2", target_bir_lowering=False)

    # ---- DRAM I/O (per-core tensors; same shapes on every core) ----
    hT = nc.dram_tensor("hT", [H, L], f32, kind="ExternalInput")        # hidden[b].T
    hTq = nc.dram_tensor("hTq", [H, ILEN], f32, kind="ExternalInput")   # hT[:, i-slice]
    hB = nc.dram_tensor("hB", [L, H], f32, kind="ExternalInput")        # hidden[b]
    WqT = nc.dram_tensor("WqT", [H, D], f32, kind="ExternalInput")
    WkT = nc.dram_tensor("WkT", [H, D], f32, kind="ExternalInput")
    bq = nc.dram_tensor("bq", [1, D], f32, kind="ExternalInput")
    bk = nc.dram_tensor("bk", [1, D], f32, kind="ExternalInput")
    wd = nc.dram_tensor("wd", [D, 1], f32, kind="ExternalInput")        # w_att[:D]
    sbias = nc.dram_tensor("sbias", [ILEN, L], f32, kind="ExternalInput")
    elm_out = nc.dram_tensor("elm_out", [ILEN, L, D], f32, kind="ExternalOutput")
    av_out = nc.dram_tensor("av_out", [ILEN, H], f32, kind="ExternalOutput")
    q_scr = nc.dram_tensor("q_scr", [1, ILEN * D], f32, kind="Internal")

    with ExitStack() as ctx:
        tc = ctx.enter_context(tile.TileContext(nc))
        const = ctx.enter_context(tc.tile_pool(name="const", bufs=1))
        sbwork = ctx.enter_context(tc.tile_pool(name="sbwork", bufs=1))
        qfpool = ctx.enter_context(tc.tile_pool(name="qf", bufs=2))
        elmpool = ctx.enter_context(tc.tile_pool(name="elm", bufs=3))
        repsb = ctx.enter_context(tc.tile_pool(name="repsb", bufs=4))
        mmps = ctx.enter_context(tc.tile_pool(name="mmps", bufs=2, space="PSUM"))
        repps = ctx.enter_context(tc.tile_pool(name="repps", bufs=3, space="PSUM"))
        avps = ctx.enter_context(tc.tile_pool(name="avps", bufs=2, space="PSUM"))
        scps = ctx.enter_context(tc.tile_pool(name="scps", bufs=1, space="PSUM"))

        # ---- constant loads ----
        hT_sb = const.tile([128, KT, L], f32)
        nc.sync.dma_start(out=hT_sb, in_=hT[:].rearrange("(t p) l -> p t l", p=128))
        hTq_sb = const.tile([128, KT, ILEN], f32)
        nc.sync.dma_start(out=hTq_sb, in_=hTq[:].rearrange("(t p) l -> p t l", p=128))
        hB_sb = const.tile([128, JT, H], f32)
        nc.sync.dma_start(out=hB_sb, in_=hB[:].rearrange("(t p) h -> p t h", p=128))
        WqT_sb = const.tile([128, KT, D], f32)
        nc.sync.dma_start(out=WqT_sb, in_=WqT[:].rearrange("(t p) d -> p t d", p=128))
        WkT_sb = const.tile([128, KT, D], f32)
        nc.sync.dma_start(out=WkT_sb, in_=WkT[:].rearrange("(t p) d -> p t d", p=128))
        bq_sb = const.tile([1, D], f32)
        nc.sync.dma_start(out=bq_sb, in_=bq[:])
        bk_sb = const.tile([1, D], f32)
        nc.sync.dma_start(out=bk_sb, in_=bk[:])
        wd_sb = const.tile([128, DT], f32)
        nc.sync.dma_start(out=wd_sb, in_=wd[:].rearrange("(t p) one -> p (t one)", p=128))
        sbias_sb = const.tile([ILEN, L], f32)
        nc.sync.dma_start(out=sbias_sb, in_=sbias[:])
        ones_sb = const.tile([1, L], f32)
        nc.vector.memset(ones_sb, 1.0)
        ident = const.tile([128, 128], f32)
        make_identity(nc, ident)

        def bcast(ap, reps):
            import concourse.bass as bass_mod
            return bass_mod.AP(
                tensor=ap.tensor,
                offset=ap.offset,
                ap=[list(ap.ap[0]), [0, reps]] + [list(a) for a in ap.ap[1:]],
            )

        def body():
            # ---- q rows for this core's i-slice: q_rd[i, d] (i on partitions) ----
            q_ps = mmps.tile([ILEN, D], f32, tag="mm")
            for t in range(KT):
                nc.tensor.matmul(q_ps, lhsT=hTq_sb[:, t, :], rhs=WqT_sb[:, t, :],
                                 start=(t == 0), stop=False)
            nc.tensor.matmul(q_ps, lhsT=ones_sb[0:1, 0:ILEN], rhs=bq_sb,
                             start=False, stop=True)
            q_rd = sbwork.tile([ILEN, D], f32, tag="q_rd")
            nc.vector.tensor_copy(q_rd, q_ps)
            # round-trip q through DRAM to get a flat single-partition copy
            nc.gpsimd.dma_start(out=q_scr[0, :].rearrange("(i d) -> i d", d=D),
                                in_=q_rd)

            # ---- k rows for the elementwise stage ----
            # Interleaved layout: partition p, slot r holds k[3p + r, :] so a
            # [128, IB, 3, D] elm tile maps to one contiguous DRAM run per
            # partition (j-major order j = 3p + r).
            k_rd = sbwork.tile([128, JT, D], f32, tag="k_rd")
            for t in range(JT):
                k_ps = mmps.tile([128, D], f32, tag="mm")
                for u in range(KT):
                    lhsT = hT_sb[:, u, :].rearrange("p (j r) -> p r j", r=JT)[:, t, :]
                    nc.tensor.matmul(k_ps, lhsT=lhsT,
                                     rhs=WkT_sb[:, u, :], start=(u == 0), stop=False)
                nc.tensor.matmul(k_ps, lhsT=ones_sb[0:1, 0:128], rhs=bk_sb,
                                 start=False, stop=True)
                nc.vector.tensor_copy(k_rd[:, t, :], k_ps)

            # ---- the big one: elmwise[i, j, d] = q[i, d] * k[j, d] ----
            for c in range(NCHUNK):
                q_flat = qfpool.tile([1, CHUNK * D], f32, tag="qf")
                nc.gpsimd.dma_start(out=q_flat,
                                    in_=q_scr[0:1, c * CHUNK * D:(c + 1) * CHUNK * D])
                for g in range(CHUNK // IB):
                    em = (elmpool.tile([128, IB, JT, D], f32, tag="em", name="em")
                          if mode != "nodve" else None)
                    for ib in range(IB):
                        ii = g * IB + ib
                        i = c * CHUNK + ii
                        rep = repps.tile([128, D], f32, tag="rep")
                        nc.tensor.matmul(rep, lhsT=ones_sb[0:1, 0:128],
                                         rhs=q_flat[0:1, ii * D:(ii + 1) * D],
                                         start=True, stop=True)
                        if mode == "nodve":
                            continue
                        if elm == "psum3":
                            nc.vector.tensor_mul(out=em[:, ib], in0=k_rd,
                                                 in1=bcast(rep, JT))
                            continue
                        # stage the broadcast row in SBUF via the idle ScalarE
                        rep_sb = repsb.tile([128, D], f32, tag="rep_sb")
                        nc.scalar.activation(
                            out=rep_sb, in_=rep,
                            func=mybir.ActivationFunctionType.Copy)
                        if elm == "dve3":
                            nc.vector.tensor_mul(out=em[:, ib], in0=k_rd,
                                                 in1=bcast(rep_sb, JT))
                        else:  # split DVE (2 slices) / GPSIMD (1)
                            nc.vector.tensor_mul(out=em[:, ib, 0:2],
                                                 in0=k_rd[:, 0:2],
                                                 in1=bcast(rep_sb, 2))
                            nc.gpsimd.tensor_mul(out=em[:, ib, 2], in0=k_rd[:, 2],
                                                 in1=rep_sb)
                    if mode == "nodma":
                        continue
                    i0g = c * CHUNK + g * IB
                    dst = elm_out[i0g:i0g + IB].rearrange(
                        "ii (p r) d -> p ii r d", r=JT)
                    src = bcast(k_rd, IB) if mode == "nodve" else em
                    eng = nc.sync if (c * (CHUNK // IB) + g) % 2 == 0 else nc.scalar
                    eng.dma_start(out=dst, in_=src)

            # ---- qT (scaled by w_att[:D]) and kT, [d, row] layout for scores ----
            qsT = sbwork.tile([128, DT, ILEN], f32, tag="qsT")
            for t in range(DT):
                qT_ps = mmps.tile([128, ILEN], f32, tag="mm")
                for u in range(KT):
                    nc.tensor.matmul(qT_ps,
                                     lhsT=WqT_sb[:, u, t * 128:(t + 1) * 128],
                                     rhs=hTq_sb[:, u, :], start=(u == 0), stop=False)
                nc.tensor.matmul(qT_ps, lhsT=bq_sb[0:1, t * 128:(t + 1) * 128],
                                 rhs=ones_sb[0:1, 0:ILEN], start=False, stop=True)
                nc.vector.tensor_scalar_mul(qsT[:, t, :], in0=qT_ps,
                                            scalar1=wd_sb[:, t:t + 1])
            kT = sbwork.tile([128, DT, L], f32, tag="kT")
            for t in range(DT):
                kT_ps = mmps.tile([128, L], f32, tag="mm")
                for u in range(KT):
                    nc.tensor.matmul(kT_ps,
                                     lhsT=WkT_sb[:, u, t * 128:(t + 1) * 128],
                                     rhs=hT_sb[:, u, :], start=(u == 0), stop=False)
                nc.tensor.matmul(kT_ps, lhsT=bk_sb[0:1, t * 128:(t + 1) * 128],
                                 rhs=ones_sb[0:1, 0:L], start=False, stop=True)
                nc.vector.tensor_copy(kT[:, t, :], kT_ps)

            # ---- scores, softmax ----
            sc_ps = scps.tile([ILEN, L], f32, tag="sc")
            for t in range(DT):
                nc.tensor.matmul(sc_ps, lhsT=qsT[:, t, :], rhs=kT[:, t, :],
                                 start=(t == 0), stop=(t == DT - 1))
            probs = sbwork.tile([ILEN, L], f32, tag="probs")
            nc.vector.tensor_add(out=probs, in0=sc_ps, in1=sbias_sb)
            negmax = sbwork.tile([ILEN, 1], f32, tag="negmax")
            nc.vector.reduce_max(out=negmax, in_=probs, axis=mybir.AxisListType.X,
                                 negate=True)
            sums = sbwork.tile([ILEN, 1], f32, tag="sums")
            nc.scalar.activation(out=probs, in_=probs,
                                 func=mybir.ActivationFunctionType.Exp,
                                 bias=negmax, scale=1.0, accum_out=sums)
            rsum = sbwork.tile([ILEN, 1], f32, tag="rsum")
            nc.vector.reciprocal(out=rsum, in_=sums)
            nc.vector.tensor_scalar_mul(out=probs, in0=probs, scalar1=rsum)

            # ---- probs.T via PE transpose, then output = probs @ hidden ----
            pT = sbwork.tile([128, JT, ILEN], f32, tag="pT")
            for t in range(JT):
                pt_ps = mmps.tile([128, ILEN], f32, tag="mm")
                nc.tensor.transpose(pt_ps, probs[:, t * 128:(t + 1) * 128],
                                    ident[0:ILEN, 0:ILEN])
                nc.vector.tensor_copy(pT[:, t, :], pt_ps)
            av_sb = sbwork.tile([ILEN, H], f32, tag="av_sb")
            for half in range(2):
                av_ps = avps.tile([ILEN, 384], f32, tag="av")
                for t in range(JT):
                    nc.tensor.matmul(av_ps, lhsT=pT[:, t, :],
                                     rhs=hB_sb[:, t, half * 384:(half + 1) * 384],
                                     start=(t == 0), stop=(t == JT - 1))
                nc.vector.tensor_copy(av_sb[:, half * 384:(half + 1) * 384], av_ps)
            nc.gpsimd.dma_start(out=av_out[:], in_=av_sb)

        if repeat >= 0:
            for _rep in range(repeat):
                body()
        else:  # hardware loop: -repeat iterations, serialized by the
               # Tile back-edge barrier (used for wall-clock timing)
            from concourse import mybir as _mb
            with tc.For_i(0, -repeat, 1, hint_engines=(
                    _mb.EngineType.PE, _mb.EngineType.DVE, _mb.EngineType.SP,
                    _mb.EngineType.Activation, _mb.EngineType.Pool)):
                body()

    nc.finalize()
    return nc


def _get_nc(repeat=1, mode="full", elm="psum3"):
    key = f"nc_{repeat}_{mode}_{elm}"
    if key not in _CACHE:
        _CACHE[key] = _build_bass(repeat, mode, elm)
    return _CACHE[key]


def _host_prep(**inputs):
    hidden = np.ascontiguousarray(np.asarray(inputs["hidden"]), dtype=np.float32)
    coo = np.asarray(inputs["dep_graph_coo"])
    etype = np.asarray(inputs["dep_graph_etype"])
    W_q = np.asarray(inputs["W_q"], dtype=np.float32)
    b_q = np.asarray(inputs["b_q"], dtype=np.float32)
    W_k = np.asarray(inputs["W_k"], dtype=np.float32)
    b_k = np.asarray(inputs["b_k"], dtype=np.float32)
    w_att = np.asarray(inputs["w_att"], dtype=np.float32)
    b_att = np.float32(np.asarray(inputs["b_att"]))
    etype_emb = np.asarray(inputs["etype_emb"], dtype=np.float32)

    # host: COO -> dense with duplicate summation (tiny, data-dependent)
    dep_graphs = np.zeros((B, L, L), dtype=np.int32)
    for b in range(B):
        np.add.at(dep_graphs[b],
                  (coo[b, 0].astype(np.int64), coo[b, 1].astype(np.int64)),
                  etype[b].astype(np.int32))
    dep_masks = (1.0 - (dep_graphs > 0)).astype(np.float32) * np.float32(-10000.0)
    # edge-type score contribution collapses to a 76-entry table lookup
    proj = (etype_emb @ w_att[D:]).astype(np.float32)
    edge_sc = np.take(proj, np.clip(dep_graphs, 0, NTYPES - 1))
    score_bias = (edge_sc + dep_masks + b_att).astype(np.float32)

    WqT = np.ascontiguousarray(W_q.T)
    WkT = np.ascontiguousarray(W_k.T)
    wd_col = np.ascontiguousarray(w_att[:D, None])
    bq_row = np.ascontiguousarray(b_q[None, :])
    bk_row = np.ascontiguousarray(b_k[None, :])

    in_maps = []
    for c in range(NCORES):
        b, i0 = c // CORES_PER_B, (c % CORES_PER_B) * ILEN
        hb = hidden[b]
        hbT = np.ascontiguousarray(hb.T)
        in_maps.append({
            "hT": hbT,
            "hTq": np.ascontiguousarray(hbT[:, i0:i0 + ILEN]),
            "hB": hb,
            "WqT": WqT,
            "WkT": WkT,
            "bq": bq_row,
            "bk": bk_row,
            "wd": wd_col,
            "sbias": np.ascontiguousarray(score_bias[b, i0:i0 + ILEN]),
        })
    return in_maps, dep_graphs, dep_masks


def make_in_maps(**inputs):
    return _host_prep(**inputs)[0]


def kernel(**inputs):
    in_maps, dep_graphs, dep_masks = _host_prep(**inputs)

    from concourse import bass_utils
    nc = _get_nc()
    res = bass_utils.run_bass_kernel_spmd(nc, in_maps, core_ids=list(range(NCORES)))
    _CACHE["last_results"] = res

    output = np.empty((B, L, H), dtype=np.float32)
    elmwise = np.empty((B, L, L, D), dtype=np.float32)
    for c in range(NCORES):
        b, i0 = c // CORES_PER_B, (c % CORES_PER_B) * ILEN
        output[b, i0:i0 + ILEN] = res.results[c]["av_out"]
        elmwise[b, i0:i0 + ILEN] = res.results[c]["elm_out"]
    return output, elmwise, dep_graphs, dep_masks
